# revision 35
# baseline (speedup 1.0000x reference)
"""Trainium2 Bass kernel for LocalAveragePoolingSegmenter (segment mean-pool).

Strategy: pure data-parallel over batch (2 batches per core on 8 cores).
Per batch, instead of the O(Tt*Ta*D) masked einsum, compute per-128-frame
local cumsums of the audio with triangular fp32 matmuls, store them to a
DRAM table, and reconstruct each token's segment sum with two indirect-DMA
row gathers plus a tiny signed-one-hot matmul against a 33-row block-offset
table. Host precomputes all index/one-hot/reciprocal tensors from
asr_alignment / text_token_len (tiny int tensors); all heavy data stays on
device.
"""

import numpy as np

B, TA, TT, D = 16, 4096, 1024, 512
NCORES = 8
BPC = B // NCORES       # batches per core
NBLK = TA // 128        # 32 time blocks per batch
NG = 4                  # audio DMA groups (8 blocks = 1024 rows each)
BLKG = NBLK // NG       # 8 blocks per group
NTOK = TT // 128        # 8 token tiles per batch
ZROW = TA               # index of the all-zero row in the P table
POFF = NBLK + 1         # 33 rows in block-offset table

_CACHE = {}


def _build_program():
    import concourse.bass as bass
    import concourse.tile as tile
    from concourse import bacc, mybir
    from concourse.tile_rust import add_dep_helper

    def _mi(x):
        return getattr(x, "ins", x)

    nc = bacc.Bacc("TRN2", target_bir_lowering=False, debug=False,
                   enable_asserts=False, num_devices=NCORES)

    f32, i32, i16, bf16 = (mybir.dt.float32, mybir.dt.int32, mybir.dt.int16,
                           mybir.dt.bfloat16)
    # audio packed as bf16 [hi(512) | lo(512)] per frame -> 2 KiB DMA rows
    ahl_in = nc.dram_tensor("audio_hl", [BPC, TA, 2 * D], bf16, kind="ExternalInput").ap()
    # gather indices, int16, wrapped [128, TT//16] (replicated per 16-part
    # group); columns 0:64 = end-row idx, 64:128 = start-row idx
    pidx_in = nc.dram_tensor("pidx", [BPC, 128, 2 * (TT // 16)], i16, kind="ExternalInput").ap()
    bt_in = nc.dram_tensor("bt", [BPC, POFF, TT], bf16, kind="ExternalInput").ap()
    recip_in = nc.dram_tensor("recip", [BPC, 128, NTOK], f32, kind="ExternalInput").ap()
    lridx_in = nc.dram_tensor("lridx", [NBLK, 1], i32, kind="ExternalInput").ap()
    ut_in = nc.dram_tensor("ut_c", [128, 128], bf16, kind="ExternalInput").ap()
    stut_in = nc.dram_tensor("stut_c", [NBLK, POFF], f32, kind="ExternalInput").ap()
    seg_out = nc.dram_tensor("seg", [BPC, TT, D], f32, kind="ExternalOutput").ap()

    with tile.TileContext(nc) as tc:
        with (
            tc.tile_pool(name="const", bufs=1) as cpool,
            tc.tile_pool(name="xg", bufs=3) as xpool,
            tc.tile_pool(name="pg", bufs=3) as pgpool,
            tc.tile_pool(name="small", bufs=2) as spool,
            tc.tile_pool(name="gath", bufs=3) as gpool,
            tc.tile_pool(name="outp", bufs=3) as opool,
            tc.tile_pool(name="ps", bufs=4, space="PSUM") as pspool,
            tc.tile_pool(name="pstok", bufs=2, space="PSUM") as pstokpool,
            tc.tile_pool(name="psoff", bufs=2, space="PSUM") as psoffpool,
            tc.tile_pool(name="pdram", bufs=2, space="DRAM") as dpool,
        ):
            # constants (host-provided; keeps gpsimd off the standard
            # library so only the mlp library is needed) -------------------
            ut = cpool.tile([128, 128], bf16)
            nc.scalar.dma_start(ut[:], ut_in[:])
            stut = cpool.tile([NBLK, POFF], f32)
            nc.scalar.dma_start(stut[:], stut_in[:])
            zrow = cpool.tile([1, D], f32)
            nc.vector.memset(zrow[:], 0.0)
            lridx = cpool.tile([NBLK, 1], i32)
            nc.scalar.dma_start(lridx[:], lridx_in[:])
            # dummy dma_gather so the auto-inserted mlp library load lands
            # at t=0 instead of right before the first real gather
            libi = cpool.tile([128, 1], i16)
            nc.vector.memset(libi[:], 0)
            libg = cpool.tile([128, 128], bf16)
            nc.gpsimd.dma_gather(
                out_ap=libg[:].rearrange("p (j d) -> p j d", j=1),
                in_ap=ut_in[:], idxs_ap=libi[:],
                num_idxs=16, num_idxs_reg=16, elem_size=128, queue_num=0,
            )

            # small per-batch inputs up front (no deps; scalar HWDGE ring)
            idxs, bts, rcs = {}, {}, {}
            for b in range(BPC):
                idx_sb = spool.tile([128, 2 * (TT // 16)], i16, tag="idx")
                nc.scalar.dma_start(idx_sb[:], pidx_in[b])
                bt_sb = spool.tile([POFF, TT], bf16, tag="bt")
                nc.scalar.dma_start(bt_sb[:], bt_in[b])
                rc_sb = spool.tile([128, NTOK], f32, tag="rc")
                nc.scalar.dma_start(rc_sb[:], recip_in[b])
                idxs[b], bts[b], rcs[b] = idx_sb, bt_sb, rc_sb

            # Software pipeline: phase A + gather issue per batch first,
            # then the combine phases. Keeps the in-order PE / sync streams
            # from stalling on batch 0's gathers before batch 1's compute.
            ptabs, offs = {}, {}
            for b in range(BPC):
                ptab = dpool.tile([TA + 1, D], f32)
                ptabs[b] = ptab
                # zero row written up front (no deps)
                nc.scalar.dma_start(ptab[ZROW : ZROW + 1, :], zrow[:])
            gathers = {}
            for b in range(BPC):
                # ---- phase A: local cumsums -> P table in DRAM ----------
                ptab = ptabs[b]
                for g in range(NG):
                    xhl = xpool.tile([128, BLKG * 2 * D], bf16, tag="xhl")
                    last_load = nc.sync.dma_start(
                        xhl[:].rearrange("p (k d) -> p k d", k=BLKG),
                        ahl_in[b, 1024 * g : 1024 * (g + 1), :]
                        .rearrange("(k p) d -> p k d", p=128),
                    )
                    pgt = pgpool.tile([128, BLKG * D], f32, tag="pg")
                    for k8 in range(BLKG):
                        psc = pspool.tile([128, D], f32, tag="psc")
                        nc.tensor.matmul(
                            out=psc[:], lhsT=ut[:],
                            rhs=xhl[:, 2 * D * k8 : 2 * D * k8 + D],
                            start=True, stop=False,
                        )
                        last_a_mm = nc.tensor.matmul(
                            out=psc[:], lhsT=ut[:],
                            rhs=xhl[:, 2 * D * k8 + D : 2 * D * (k8 + 1)],
                            start=False, stop=True,
                        )
                        nc.scalar.copy(pgt[:, bass.ts(k8, D)], psc[:])
                    # P writes on the scalar HWDGE ring so they never block
                    # the next batch's loads on the sync ring
                    nc.scalar.dma_start(
                        ptab[1024 * g : 1024 * (g + 1), :]
                        .rearrange("(k p) d -> p k d", p=128),
                        pgt[:].rearrange("p (k d) -> p k d", k=BLKG),
                    )

                # ---- gathers for this batch (lastrows + 4 half-gathers) --
                idx_sb = idxs[b]
                lrows = spool.tile([NBLK, D], f32, tag="lrows")
                lrows_i = nc.gpsimd.indirect_dma_start(
                    out=lrows[:], out_offset=None, in_=ptab[:],
                    in_offset=bass.IndirectOffsetOnAxis(ap=lridx[:, :1], axis=0),
                )
                if b > 0:
                    add_dep_helper(_mi(lrows_i), _mi(last_gather), sync=False,
                                   reason="keep gather phases batch-ordered")
                HT = TT // 2          # 512 idx per half-gather
                HC = HT // 16         # idx columns per half
                HJ = NTOK // 2        # token tiles per half
                halves = []
                for h in range(2):
                    geh = gpool.tile([128, HJ * D], f32, tag=f"ge{h}", bufs=2)
                    nc.gpsimd.dma_gather(
                        out_ap=geh[:].rearrange("p (j d) -> p j d", j=HJ),
                        in_ap=ptab[:], idxs_ap=idx_sb[:, HC * h : HC * (h + 1)],
                        num_idxs=HT, num_idxs_reg=HT, elem_size=D, queue_num=0,
                    )
                    gsh = gpool.tile([128, HJ * D], f32, tag=f"gs{h}", bufs=2)
                    last_gather = nc.gpsimd.dma_gather(
                        out_ap=gsh[:].rearrange("p (j d) -> p j d", j=HJ),
                        in_ap=ptab[:],
                        idxs_ap=idx_sb[:, HC * (2 + h) : HC * (3 + h)],
                        num_idxs=HT, num_idxs_reg=HT, elem_size=D, queue_num=0,
                    )
                    halves.append((geh, gsh))
                gathers[b] = (lrows, halves)

            for b in range(BPC):
                lrows, halves = gathers[b]
                bt_sb, rc_sb = bts[b], rcs[b]
                # block-offset table off[33, D] and its bf16 hi/lo split
                psoff = psoffpool.tile([POFF, D], f32, tag="psoff")
                psoff_mm = nc.tensor.matmul(out=psoff[:], lhsT=stut[:],
                                            rhs=lrows[:], start=True, stop=True)
                add_dep_helper(_mi(psoff_mm), _mi(last_a_mm), sync=False,
                               reason="phase-B PE ops after phase-A matmuls")
                off_sb = spool.tile([POFF, D], f32, tag="off")
                nc.vector.tensor_copy(off_sb[:], psoff[:])
                off_hi = spool.tile([POFF, D], bf16, tag="offh")
                nc.vector.tensor_copy(off_hi[:], off_sb[:])
                off_hf = spool.tile([POFF, D], f32, tag="offhf")
                nc.vector.tensor_copy(off_hf[:], off_hi[:])
                off_lo = spool.tile([POFF, D], bf16, tag="offl")
                nc.vector.tensor_sub(off_lo[:], off_sb[:], off_hf[:])

                HJ = NTOK // 2
                for j in range(NTOK):
                    geh, gsh = halves[j // HJ]
                    jj = j % HJ
                    pstok = pstokpool.tile([128, D], f32, tag="pstok")
                    pst_mm = nc.tensor.matmul(out=pstok[:],
                                              lhsT=bt_sb[:, bass.ts(j, 128)],
                                              rhs=off_hi[:], start=True, stop=False)
                    add_dep_helper(_mi(pst_mm), _mi(last_a_mm), sync=False,
                                   reason="phase-B PE ops after phase-A matmuls")
                    nc.tensor.matmul(out=pstok[:],
                                     lhsT=bt_sb[:, bass.ts(j, 128)],
                                     rhs=off_lo[:], start=False, stop=True)
                    d1 = gpool.tile([128, D], f32, tag="d1")
                    nc.vector.tensor_sub(d1[:], geh[:, bass.ts(jj, D)],
                                         gsh[:, bass.ts(jj, D)])
                    nc.vector.tensor_add(d1[:], d1[:], pstok[:])
                    ot = opool.tile([128, D], f32, tag="ot")
                    nc.vector.tensor_scalar_mul(ot[:], d1[:], rc_sb[:, j : j + 1])
                    ow = nc.sync.dma_start(seg_out[b, bass.ts(j, 128), :], ot[:])
                    add_dep_helper(_mi(ow), _mi(last_load), sync=False,
                                   reason="out writes after all loads on sync ring")

    nc.compile()
    return nc


def _get_program():
    if "nc" not in _CACHE:
        _CACHE["nc"] = _build_program()
    return _CACHE["nc"]


def _host_prep(asr_alignment, text_token_len):
    import ml_dtypes
    a = np.asarray(asr_alignment).astype(np.int64)
    s, e = a[..., 0], a[..., 1]
    tlen = np.asarray(text_token_len).astype(np.int64)
    tmask = np.arange(TT)[None, :] < tlen[:, None]

    pe = np.where(((e + 1) % 128) != 0, e, ZROW)
    ps = np.where((s % 128) != 0, s - 1, ZROW)
    pe = np.where(tmask, pe, ZROW)
    ps = np.where(tmask, ps, ZROW)
    be = np.where(tmask, (e + 1) >> 7, 0)
    bs = np.where(tmask, s >> 7, 0)

    cnt = (e - s + 1).astype(np.float64)
    recip = np.where(tmask, 1.0 / cnt, 0.0).astype(np.float32)

    # dma_gather wrap layout per 512-token half: flat token k at
    # [k % 16, k // 16], replicated across the 8 Q7-core partition groups.
    # Column blocks: [pe half0 | pe half1 | ps half0 | ps half1]
    def wrap(v):  # v: [B, 512] -> [B, 128, 32] int16
        w = v.reshape(B, 32, 16).transpose(0, 2, 1).astype(np.int16)
        return np.tile(w, (1, 8, 1))

    H = TT // 2
    pidx = np.concatenate([wrap(pe[:, :H]), wrap(pe[:, H:]),
                           wrap(ps[:, :H]), wrap(ps[:, H:])], axis=2)
    pidx = np.ascontiguousarray(pidx)

    k = np.arange(128)
    ut_c = (k[:, None] <= k[None, :]).astype(ml_dtypes.bfloat16)
    j32 = np.arange(NBLK)[:, None]
    stut_c = (j32 < np.arange(POFF)[None, :]).astype(np.float32)

    ks = np.arange(POFF)[None, :, None]
    bt = ((ks == be[:, None, :]).astype(np.float32)
          - (ks == bs[:, None, :]).astype(np.float32)).astype(ml_dtypes.bfloat16)

    recip_dev = recip.reshape(B, NTOK, 128).transpose(0, 2, 1)
    recip_dev = np.ascontiguousarray(recip_dev, np.float32)
    return pidx, np.ascontiguousarray(bt), recip_dev, ut_c, stut_c


def _run(inputs_by_core, trace=False, **kw):
    from concourse.bass_utils import run_bass_kernel_spmd
    nc = _get_program()
    return run_bass_kernel_spmd(nc, inputs_by_core,
                                core_ids=list(range(NCORES)), trace=trace, **kw)


def _make_in_maps(audio_feats, asr_alignment, text_token_len):
    import ml_dtypes
    audio = np.ascontiguousarray(np.asarray(audio_feats), np.float32)
    audio_hl = np.empty((B, TA, 2 * D), ml_dtypes.bfloat16)
    audio_hl[:, :, :D] = audio.astype(ml_dtypes.bfloat16)
    audio_hl[:, :, D:] = (audio - audio_hl[:, :, :D].astype(np.float32)
                          ).astype(ml_dtypes.bfloat16)
    pidx, bt, recip, ut_c, stut_c = _host_prep(asr_alignment, text_token_len)
    lridx = (np.arange(NBLK, dtype=np.int32) * 128 + 127).reshape(NBLK, 1)
    in_maps = []
    for c in range(NCORES):
        sl = slice(BPC * c, BPC * (c + 1))
        in_maps.append({
            "audio_hl": audio_hl[sl],
            "pidx": pidx[sl],
            "bt": bt[sl],
            "recip": recip[sl],
            "lridx": lridx,
            "ut_c": ut_c,
            "stut_c": stut_c,
        })
    return in_maps


def kernel(audio_feats, audio_feats_len, text_token_for_audio,
           text_token_embed_for_audio, text_token_len, asr_alignment,
           _trace=False, **_kw):
    in_maps = _make_in_maps(audio_feats, asr_alignment, text_token_len)
    res = _run(in_maps, trace=_trace, **_kw)
    seg = np.concatenate([res.results[c]["seg"] for c in range(NCORES)], axis=0)
    out_len = np.asarray(text_token_len).astype(np.int32, copy=False)
    if _trace:
        return (seg, out_len), res
    return seg, out_len


# revision 39
# speedup vs baseline: 1.0404x; 1.0404x over previous
"""Trainium2 Bass kernel for LocalAveragePoolingSegmenter (segment mean-pool).

Strategy: pure data-parallel over batch (2 batches per core on 8 cores).
Per batch, instead of the O(Tt*Ta*D) masked einsum, compute per-128-frame
local cumsums of the audio with triangular fp32 matmuls, store them to a
DRAM table, and reconstruct each token's segment sum with two indirect-DMA
row gathers plus a tiny signed-one-hot matmul against a 33-row block-offset
table. Host precomputes all index/one-hot/reciprocal tensors from
asr_alignment / text_token_len (tiny int tensors); all heavy data stays on
device.
"""

import numpy as np

B, TA, TT, D = 16, 4096, 1024, 512
NCORES = 8
BPC = B // NCORES       # batches per core
NBLK = TA // 128        # 32 time blocks per batch
NG = 4                  # audio DMA groups (8 blocks = 1024 rows each)
BLKG = NBLK // NG       # 8 blocks per group
NTOK = TT // 128        # 8 token tiles per batch
ZROW = TA               # index of the all-zero row in the P table
POFF = NBLK + 1         # 33 rows in block-offset table

_CACHE = {}


def _build_program():
    import concourse.bass as bass
    import concourse.tile as tile
    from concourse import bacc, mybir
    from concourse.tile_rust import add_dep_helper

    def _mi(x):
        return getattr(x, "ins", x)

    nc = bacc.Bacc("TRN2", target_bir_lowering=False, debug=False,
                   enable_asserts=False, num_devices=NCORES)

    f32, i32, i16, bf16 = (mybir.dt.float32, mybir.dt.int32, mybir.dt.int16,
                           mybir.dt.bfloat16)
    # audio packed as bf16 [hi(512) | lo(512)] per frame -> 2 KiB DMA rows
    ahl_in = nc.dram_tensor("audio_hl", [BPC, TA, 2 * D], bf16, kind="ExternalInput").ap()
    # gather indices, int16, wrapped [128, TT//16] (replicated per 16-part
    # group); columns 0:64 = end-row idx, 64:128 = start-row idx
    pidx_in = nc.dram_tensor("pidx", [BPC, 128, 2 * (TT // 16)], i16, kind="ExternalInput").ap()
    bt_in = nc.dram_tensor("bt", [BPC, POFF, TT], bf16, kind="ExternalInput").ap()
    recip_in = nc.dram_tensor("recip", [BPC, 128, NTOK], f32, kind="ExternalInput").ap()
    lridx_in = nc.dram_tensor("lridx", [NBLK, 1], i32, kind="ExternalInput").ap()
    ut_in = nc.dram_tensor("ut_c", [128, 128], bf16, kind="ExternalInput").ap()
    stut_in = nc.dram_tensor("stut_c", [NBLK, POFF], f32, kind="ExternalInput").ap()
    seg_out = nc.dram_tensor("seg", [BPC, TT, D], f32, kind="ExternalOutput").ap()

    with tile.TileContext(nc) as tc:
        with (
            tc.tile_pool(name="const", bufs=1) as cpool,
            tc.tile_pool(name="xg", bufs=3) as xpool,
            tc.tile_pool(name="pg", bufs=3) as pgpool,
            tc.tile_pool(name="small", bufs=2) as spool,
            tc.tile_pool(name="gath", bufs=3) as gpool,
            tc.tile_pool(name="outp", bufs=3) as opool,
            tc.tile_pool(name="ps", bufs=4, space="PSUM") as pspool,
            tc.tile_pool(name="pstok", bufs=2, space="PSUM") as pstokpool,
            tc.tile_pool(name="psoff", bufs=2, space="PSUM") as psoffpool,
            tc.tile_pool(name="pdram", bufs=2, space="DRAM") as dpool,
        ):
            # constants (host-provided; keeps gpsimd off the standard
            # library so only the mlp library is needed) -------------------
            ut = cpool.tile([128, 128], bf16)
            nc.scalar.dma_start(ut[:], ut_in[:])
            stut = cpool.tile([NBLK, POFF], f32)
            nc.scalar.dma_start(stut[:], stut_in[:])
            zrow = cpool.tile([1, D], f32)
            nc.vector.memset(zrow[:], 0.0)
            lridx = cpool.tile([NBLK, 1], i32)
            nc.scalar.dma_start(lridx[:], lridx_in[:])
            # dummy dma_gather so the auto-inserted mlp library load lands
            # at t=0 instead of right before the first real gather
            libi = cpool.tile([128, 1], i16)
            nc.vector.memset(libi[:], 0)
            libg = cpool.tile([128, 128], bf16)
            nc.gpsimd.dma_gather(
                out_ap=libg[:].rearrange("p (j d) -> p j d", j=1),
                in_ap=ut_in[:], idxs_ap=libi[:],
                num_idxs=16, num_idxs_reg=16, elem_size=128, queue_num=0,
            )

            # small per-batch inputs up front (no deps; scalar HWDGE ring)
            idxs, bts, rcs = {}, {}, {}
            for b in range(BPC):
                idx_sb = spool.tile([128, 2 * (TT // 16)], i16, tag="idx")
                nc.scalar.dma_start(idx_sb[:], pidx_in[b])
                bt_sb = spool.tile([POFF, TT], bf16, tag="bt")
                nc.scalar.dma_start(bt_sb[:], bt_in[b])
                rc_sb = spool.tile([128, NTOK], f32, tag="rc")
                nc.scalar.dma_start(rc_sb[:], recip_in[b])
                idxs[b], bts[b], rcs[b] = idx_sb, bt_sb, rc_sb

            # Software pipeline: phase A + gather issue per batch first,
            # then the combine phases. Keeps the in-order PE / sync streams
            # from stalling on batch 0's gathers before batch 1's compute.
            ptabs, offs, lrows_all = {}, {}, {}
            for b in range(BPC):
                ptab = dpool.tile([TA + 1, D], f32)
                ptabs[b] = ptab
                # zero row written up front (no deps)
                nc.scalar.dma_start(ptab[ZROW : ZROW + 1, :], zrow[:])
            gathers = {}
            for b in range(BPC):
                # ---- phase A: local cumsums -> P table in DRAM ----------
                ptab = ptabs[b]
                pwrites = []
                for g in range(NG):
                    xhl = xpool.tile([128, BLKG * 2 * D], bf16, tag="xhl")
                    last_load = nc.sync.dma_start(
                        xhl[:].rearrange("p (k d) -> p k d", k=BLKG),
                        ahl_in[b, 1024 * g : 1024 * (g + 1), :]
                        .rearrange("(k p) d -> p k d", p=128),
                    )
                    pgt = pgpool.tile([128, BLKG * D], f32, tag="pg")
                    for k8 in range(BLKG):
                        psc = pspool.tile([128, D], f32, tag="psc")
                        nc.tensor.matmul(
                            out=psc[:], lhsT=ut[:],
                            rhs=xhl[:, 2 * D * k8 : 2 * D * k8 + D],
                            start=True, stop=False,
                        )
                        last_a_mm = nc.tensor.matmul(
                            out=psc[:], lhsT=ut[:],
                            rhs=xhl[:, 2 * D * k8 + D : 2 * D * (k8 + 1)],
                            start=False, stop=True,
                        )
                        nc.scalar.copy(pgt[:, bass.ts(k8, D)], psc[:])
                    # P writes on the scalar HWDGE ring so they never block
                    # the next batch's loads on the sync ring
                    pw = nc.scalar.dma_start(
                        ptab[1024 * g : 1024 * (g + 1), :]
                        .rearrange("(k p) d -> p k d", p=128),
                        pgt[:].rearrange("p (k d) -> p k d", k=BLKG),
                    )
                    pwrites.append(pw)

                # lastrow fetch: static strided read (rows 127, 255, ...).
                # Tile's range tracking misses this strided-read/ rearranged-
                # write overlap, so add the write deps explicitly.
                lrows = spool.tile([NBLK, D], f32, tag="lrows")
                lr_dma = nc.scalar.dma_start(
                    lrows[:],
                    ptab[:TA, :].rearrange("(k p) d -> k p d", p=128)[:, 127, :],
                )
                for pw in pwrites:
                    add_dep_helper(_mi(lr_dma), _mi(pw), sync=True,
                                   reason="lastrow read after P writes")
                lrows_all[b] = lrows

                # ---- gathers for this batch (4 half-gathers) -------------
                idx_sb = idxs[b]
                HT = TT // 2          # 512 idx per half-gather
                HC = HT // 16         # idx columns per half
                HJ = NTOK // 2        # token tiles per half
                halves = []
                for h in range(2):
                    geh = gpool.tile([128, HJ * D], f32, tag=f"ge{h}", bufs=2)
                    nc.gpsimd.dma_gather(
                        out_ap=geh[:].rearrange("p (j d) -> p j d", j=HJ),
                        in_ap=ptab[:], idxs_ap=idx_sb[:, HC * h : HC * (h + 1)],
                        num_idxs=HT, num_idxs_reg=HT, elem_size=D, queue_num=0,
                    )
                    gsh = gpool.tile([128, HJ * D], f32, tag=f"gs{h}", bufs=2)
                    last_gather = nc.gpsimd.dma_gather(
                        out_ap=gsh[:].rearrange("p (j d) -> p j d", j=HJ),
                        in_ap=ptab[:],
                        idxs_ap=idx_sb[:, HC * (2 + h) : HC * (3 + h)],
                        num_idxs=HT, num_idxs_reg=HT, elem_size=D, queue_num=0,
                    )
                    halves.append((geh, gsh))
                gathers[b] = (lrows, halves)

            for b in range(BPC):
                lrows, halves = gathers[b]
                bt_sb, rc_sb = bts[b], rcs[b]
                # block-offset table off[33, D] and its bf16 hi/lo split
                psoff = psoffpool.tile([POFF, D], f32, tag="psoff")
                psoff_mm = nc.tensor.matmul(out=psoff[:], lhsT=stut[:],
                                            rhs=lrows[:], start=True, stop=True)
                add_dep_helper(_mi(psoff_mm), _mi(last_a_mm), sync=False,
                               reason="phase-B PE ops after phase-A matmuls")
                off_sb = spool.tile([POFF, D], f32, tag="off")
                nc.vector.tensor_copy(off_sb[:], psoff[:])
                off_hi = spool.tile([POFF, D], bf16, tag="offh")
                nc.vector.tensor_copy(off_hi[:], off_sb[:])
                off_hf = spool.tile([POFF, D], f32, tag="offhf")
                nc.vector.tensor_copy(off_hf[:], off_hi[:])
                off_lo = spool.tile([POFF, D], bf16, tag="offl")
                nc.vector.tensor_sub(off_lo[:], off_sb[:], off_hf[:])

                HJ = NTOK // 2
                for j in range(NTOK):
                    geh, gsh = halves[j // HJ]
                    jj = j % HJ
                    pstok = pstokpool.tile([128, D], f32, tag="pstok")
                    pst_mm = nc.tensor.matmul(out=pstok[:],
                                              lhsT=bt_sb[:, bass.ts(j, 128)],
                                              rhs=off_hi[:], start=True, stop=False)
                    add_dep_helper(_mi(pst_mm), _mi(last_a_mm), sync=False,
                                   reason="phase-B PE ops after phase-A matmuls")
                    nc.tensor.matmul(out=pstok[:],
                                     lhsT=bt_sb[:, bass.ts(j, 128)],
                                     rhs=off_lo[:], start=False, stop=True)
                    d1 = gpool.tile([128, D], f32, tag="d1")
                    nc.vector.tensor_sub(d1[:], geh[:, bass.ts(jj, D)],
                                         gsh[:, bass.ts(jj, D)])
                    nc.vector.tensor_add(d1[:], d1[:], pstok[:])
                    ot = opool.tile([128, D], f32, tag="ot")
                    nc.vector.tensor_scalar_mul(ot[:], d1[:], rc_sb[:, j : j + 1])
                    ow = nc.sync.dma_start(seg_out[b, bass.ts(j, 128), :], ot[:])
                    add_dep_helper(_mi(ow), _mi(last_load), sync=False,
                                   reason="out writes after all loads on sync ring")

    nc.compile()
    return nc


def _get_program():
    if "nc" not in _CACHE:
        _CACHE["nc"] = _build_program()
    return _CACHE["nc"]


def _host_prep(asr_alignment, text_token_len):
    import ml_dtypes
    a = np.asarray(asr_alignment).astype(np.int64)
    s, e = a[..., 0], a[..., 1]
    tlen = np.asarray(text_token_len).astype(np.int64)
    tmask = np.arange(TT)[None, :] < tlen[:, None]

    pe = np.where(((e + 1) % 128) != 0, e, ZROW)
    ps = np.where((s % 128) != 0, s - 1, ZROW)
    pe = np.where(tmask, pe, ZROW)
    ps = np.where(tmask, ps, ZROW)
    be = np.where(tmask, (e + 1) >> 7, 0)
    bs = np.where(tmask, s >> 7, 0)

    cnt = (e - s + 1).astype(np.float64)
    recip = np.where(tmask, 1.0 / cnt, 0.0).astype(np.float32)

    # dma_gather wrap layout per 512-token half: flat token k at
    # [k % 16, k // 16], replicated across the 8 Q7-core partition groups.
    # Column blocks: [pe half0 | pe half1 | ps half0 | ps half1]
    def wrap(v):  # v: [B, 512] -> [B, 128, 32] int16
        w = v.reshape(B, 32, 16).transpose(0, 2, 1).astype(np.int16)
        return np.tile(w, (1, 8, 1))

    H = TT // 2
    pidx = np.concatenate([wrap(pe[:, :H]), wrap(pe[:, H:]),
                           wrap(ps[:, :H]), wrap(ps[:, H:])], axis=2)
    pidx = np.ascontiguousarray(pidx)

    k = np.arange(128)
    ut_c = (k[:, None] <= k[None, :]).astype(ml_dtypes.bfloat16)
    j32 = np.arange(NBLK)[:, None]
    stut_c = (j32 < np.arange(POFF)[None, :]).astype(np.float32)

    ks = np.arange(POFF)[None, :, None]
    bt = ((ks == be[:, None, :]).astype(np.float32)
          - (ks == bs[:, None, :]).astype(np.float32)).astype(ml_dtypes.bfloat16)

    recip_dev = recip.reshape(B, NTOK, 128).transpose(0, 2, 1)
    recip_dev = np.ascontiguousarray(recip_dev, np.float32)
    return pidx, np.ascontiguousarray(bt), recip_dev, ut_c, stut_c


def _run(inputs_by_core, trace=False, **kw):
    from concourse.bass_utils import run_bass_kernel_spmd
    nc = _get_program()
    return run_bass_kernel_spmd(nc, inputs_by_core,
                                core_ids=list(range(NCORES)), trace=trace, **kw)


def _make_in_maps(audio_feats, asr_alignment, text_token_len):
    import ml_dtypes
    audio = np.ascontiguousarray(np.asarray(audio_feats), np.float32)
    audio_hl = np.empty((B, TA, 2 * D), ml_dtypes.bfloat16)
    audio_hl[:, :, :D] = audio.astype(ml_dtypes.bfloat16)
    audio_hl[:, :, D:] = (audio - audio_hl[:, :, :D].astype(np.float32)
                          ).astype(ml_dtypes.bfloat16)
    pidx, bt, recip, ut_c, stut_c = _host_prep(asr_alignment, text_token_len)
    lridx = (np.arange(NBLK, dtype=np.int32) * 128 + 127).reshape(NBLK, 1)
    in_maps = []
    for c in range(NCORES):
        sl = slice(BPC * c, BPC * (c + 1))
        in_maps.append({
            "audio_hl": audio_hl[sl],
            "pidx": pidx[sl],
            "bt": bt[sl],
            "recip": recip[sl],
            "lridx": lridx,
            "ut_c": ut_c,
            "stut_c": stut_c,
        })
    return in_maps


def kernel(audio_feats, audio_feats_len, text_token_for_audio,
           text_token_embed_for_audio, text_token_len, asr_alignment,
           _trace=False, **_kw):
    in_maps = _make_in_maps(audio_feats, asr_alignment, text_token_len)
    res = _run(in_maps, trace=_trace, **_kw)
    seg = np.concatenate([res.results[c]["seg"] for c in range(NCORES)], axis=0)
    out_len = np.asarray(text_token_len).astype(np.int32, copy=False)
    if _trace:
        return (seg, out_len), res
    return seg, out_len


# revision 41
# speedup vs baseline: 1.0700x; 1.0284x over previous
"""Trainium2 Bass kernel for LocalAveragePoolingSegmenter (segment mean-pool).

Strategy: pure data-parallel over batch (2 batches per core on 8 cores).
Per batch, instead of the O(Tt*Ta*D) masked einsum, compute per-128-frame
local cumsums of the audio with triangular fp32 matmuls, store them to a
DRAM table, and reconstruct each token's segment sum with two indirect-DMA
row gathers plus a tiny signed-one-hot matmul against a 33-row block-offset
table. Host precomputes all index/one-hot/reciprocal tensors from
asr_alignment / text_token_len (tiny int tensors); all heavy data stays on
device.
"""

import numpy as np

B, TA, TT, D = 16, 4096, 1024, 512
NCORES = 8
BPC = B // NCORES       # batches per core
NBLK = TA // 128        # 32 time blocks per batch
NG = 4                  # audio DMA groups (8 blocks = 1024 rows each)
BLKG = NBLK // NG       # 8 blocks per group
NTOK = TT // 128        # 8 token tiles per batch
ZROW = TA               # index of the all-zero row in the P table
POFF = NBLK + 1         # 33 rows in block-offset table

_CACHE = {}


def _build_program():
    import concourse.bass as bass
    import concourse.tile as tile
    from concourse import bacc, mybir
    from concourse.tile_rust import add_dep_helper

    def _mi(x):
        return getattr(x, "ins", x)

    nc = bacc.Bacc("TRN2", target_bir_lowering=False, debug=False,
                   enable_asserts=False, num_devices=NCORES)

    f32, i32, i16, bf16 = (mybir.dt.float32, mybir.dt.int32, mybir.dt.int16,
                           mybir.dt.bfloat16)
    # audio packed as bf16 [hi(512) | lo(512)] per frame -> 2 KiB DMA rows
    ahl_in = nc.dram_tensor("audio_hl", [BPC, TA, 2 * D], bf16, kind="ExternalInput").ap()
    # gather indices, int16, wrapped [128, TT//16] (replicated per 16-part
    # group); columns 0:64 = end-row idx, 64:128 = start-row idx
    pidx_in = nc.dram_tensor("pidx", [BPC, 128, 2 * (TT // 16)], i16, kind="ExternalInput").ap()
    bt_in = nc.dram_tensor("bt", [BPC, POFF, TT], bf16, kind="ExternalInput").ap()
    recip_in = nc.dram_tensor("recip", [BPC, 128, NTOK], f32, kind="ExternalInput").ap()
    lridx_in = nc.dram_tensor("lridx", [NBLK, 1], i32, kind="ExternalInput").ap()
    ut_in = nc.dram_tensor("ut_c", [128, 128], bf16, kind="ExternalInput").ap()
    stut_in = nc.dram_tensor("stut_c", [NBLK, POFF], f32, kind="ExternalInput").ap()
    seg_out = nc.dram_tensor("seg", [BPC, TT, D], f32, kind="ExternalOutput").ap()

    with tile.TileContext(nc) as tc:
        with (
            tc.tile_pool(name="const", bufs=1) as cpool,
            tc.tile_pool(name="xg", bufs=3) as xpool,
            tc.tile_pool(name="pg", bufs=3) as pgpool,
            tc.tile_pool(name="small", bufs=2) as spool,
            tc.tile_pool(name="gath", bufs=3) as gpool,
            tc.tile_pool(name="outp", bufs=3) as opool,
            tc.tile_pool(name="ps", bufs=4, space="PSUM") as pspool,
            tc.tile_pool(name="pstok", bufs=2, space="PSUM") as pstokpool,
            tc.tile_pool(name="psoff", bufs=2, space="PSUM") as psoffpool,
            tc.tile_pool(name="pdram", bufs=2, space="DRAM") as dpool,
        ):
            # constants (host-provided; keeps gpsimd off the standard
            # library so only the mlp library is needed) -------------------
            ut = cpool.tile([128, 128], bf16)
            nc.scalar.dma_start(ut[:], ut_in[:])
            stut = cpool.tile([NBLK, POFF], f32)
            nc.scalar.dma_start(stut[:], stut_in[:])
            zrow = cpool.tile([1, D], f32)
            nc.vector.memset(zrow[:], 0.0)
            lridx = cpool.tile([NBLK, 1], i32)
            nc.scalar.dma_start(lridx[:], lridx_in[:])
            # dummy dma_gather so the auto-inserted mlp library load lands
            # at t=0 instead of right before the first real gather
            libi = cpool.tile([128, 1], i16)
            nc.vector.memset(libi[:], 0)
            libg = cpool.tile([128, 128], bf16)
            nc.gpsimd.dma_gather(
                out_ap=libg[:].rearrange("p (j d) -> p j d", j=1),
                in_ap=ut_in[:], idxs_ap=libi[:],
                num_idxs=16, num_idxs_reg=16, elem_size=128, queue_num=0,
            )

            # small per-batch inputs up front (no deps; scalar HWDGE ring)
            idxs, bts, rcs = {}, {}, {}
            for b in range(BPC):
                idx_sb = spool.tile([128, 2 * (TT // 16)], i16, tag="idx")
                nc.scalar.dma_start(idx_sb[:], pidx_in[b])
                bt_sb = spool.tile([POFF, TT], bf16, tag="bt")
                nc.scalar.dma_start(bt_sb[:], bt_in[b])
                rc_sb = spool.tile([128, NTOK], f32, tag="rc")
                nc.scalar.dma_start(rc_sb[:], recip_in[b])
                idxs[b], bts[b], rcs[b] = idx_sb, bt_sb, rc_sb

            # Software pipeline: phase A + gather issue per batch first,
            # then the combine phases. Keeps the in-order PE / sync streams
            # from stalling on batch 0's gathers before batch 1's compute.
            ptabs, offs, lrows_all = {}, {}, {}
            for b in range(BPC):
                ptab = dpool.tile([TA + 1, D], f32)
                ptabs[b] = ptab
                # zero row written up front (no deps)
                nc.scalar.dma_start(ptab[ZROW : ZROW + 1, :], zrow[:])
            gathers = {}
            for b in range(BPC):
                # ---- phase A: local cumsums -> P table in DRAM ----------
                ptab = ptabs[b]
                lrows = spool.tile([NBLK, D], f32, tag="lrows")
                for g in range(NG):
                    xhl = xpool.tile([128, BLKG * 2 * D], bf16, tag="xhl")
                    last_load = nc.sync.dma_start(
                        xhl[:].rearrange("p (k d) -> p k d", k=BLKG),
                        ahl_in[b, 1024 * g : 1024 * (g + 1), :]
                        .rearrange("(k p) d -> p k d", p=128),
                    )
                    pgt = pgpool.tile([128, BLKG * D], f32, tag="pg")
                    for k8 in range(BLKG):
                        psc = pspool.tile([128, D], f32, tag="psc")
                        nc.tensor.matmul(
                            out=psc[:], lhsT=ut[:],
                            rhs=xhl[:, 2 * D * k8 : 2 * D * k8 + D],
                            start=True, stop=False,
                        )
                        last_a_mm = nc.tensor.matmul(
                            out=psc[:], lhsT=ut[:],
                            rhs=xhl[:, 2 * D * k8 + D : 2 * D * (k8 + 1)],
                            start=False, stop=True,
                        )
                        nc.scalar.copy(pgt[:, bass.ts(k8, D)], psc[:])
                    # P writes on the scalar HWDGE ring so they never block
                    # the next batch's loads on the sync ring
                    nc.scalar.dma_start(
                        ptab[1024 * g : 1024 * (g + 1), :]
                        .rearrange("(k p) d -> p k d", p=128),
                        pgt[:].rearrange("p (k d) -> p k d", k=BLKG),
                    )
                    # lastrows of this group straight out of SBUF (partition
                    # 127 of pgt holds each block's final cumsum row)
                    nc.scalar.dma_start(
                        lrows[BLKG * g : BLKG * (g + 1), :],
                        pgt[127:128, :].rearrange("p (k d) -> p k d", k=BLKG),
                    )
                lrows_all[b] = lrows

                # ---- gathers for this batch (4 half-gathers) -------------
                idx_sb = idxs[b]
                HT = TT // 2          # 512 idx per half-gather
                HC = HT // 16         # idx columns per half
                HJ = NTOK // 2        # token tiles per half
                halves = []
                for h in range(2):
                    geh = gpool.tile([128, HJ * D], f32, tag=f"ge{h}", bufs=2)
                    nc.gpsimd.dma_gather(
                        out_ap=geh[:].rearrange("p (j d) -> p j d", j=HJ),
                        in_ap=ptab[:], idxs_ap=idx_sb[:, HC * h : HC * (h + 1)],
                        num_idxs=HT, num_idxs_reg=HT, elem_size=D, queue_num=0,
                    )
                    gsh = gpool.tile([128, HJ * D], f32, tag=f"gs{h}", bufs=2)
                    last_gather = nc.gpsimd.dma_gather(
                        out_ap=gsh[:].rearrange("p (j d) -> p j d", j=HJ),
                        in_ap=ptab[:],
                        idxs_ap=idx_sb[:, HC * (2 + h) : HC * (3 + h)],
                        num_idxs=HT, num_idxs_reg=HT, elem_size=D, queue_num=0,
                    )
                    halves.append((geh, gsh))
                gathers[b] = (lrows, halves)

            for b in range(BPC):
                lrows, halves = gathers[b]
                bt_sb, rc_sb = bts[b], rcs[b]
                # block-offset table off[33, D] and its bf16 hi/lo split
                psoff = psoffpool.tile([POFF, D], f32, tag="psoff")
                psoff_mm = nc.tensor.matmul(out=psoff[:], lhsT=stut[:],
                                            rhs=lrows[:], start=True, stop=True)
                add_dep_helper(_mi(psoff_mm), _mi(last_a_mm), sync=False,
                               reason="phase-B PE ops after phase-A matmuls")
                off_sb = spool.tile([POFF, D], f32, tag="off")
                nc.vector.tensor_copy(off_sb[:], psoff[:])
                off_hi = spool.tile([POFF, D], bf16, tag="offh")
                nc.vector.tensor_copy(off_hi[:], off_sb[:])
                off_hf = spool.tile([POFF, D], f32, tag="offhf")
                nc.vector.tensor_copy(off_hf[:], off_hi[:])
                off_lo = spool.tile([POFF, D], bf16, tag="offl")
                nc.vector.tensor_sub(off_lo[:], off_sb[:], off_hf[:])

                HJ = NTOK // 2
                for j in range(NTOK):
                    geh, gsh = halves[j // HJ]
                    jj = j % HJ
                    pstok = pstokpool.tile([128, D], f32, tag="pstok")
                    pst_mm = nc.tensor.matmul(out=pstok[:],
                                              lhsT=bt_sb[:, bass.ts(j, 128)],
                                              rhs=off_hi[:], start=True, stop=False)
                    add_dep_helper(_mi(pst_mm), _mi(last_a_mm), sync=False,
                                   reason="phase-B PE ops after phase-A matmuls")
                    nc.tensor.matmul(out=pstok[:],
                                     lhsT=bt_sb[:, bass.ts(j, 128)],
                                     rhs=off_lo[:], start=False, stop=True)
                    d1 = gpool.tile([128, D], f32, tag="d1")
                    nc.vector.tensor_sub(d1[:], geh[:, bass.ts(jj, D)],
                                         gsh[:, bass.ts(jj, D)])
                    nc.vector.tensor_add(d1[:], d1[:], pstok[:])
                    ot = opool.tile([128, D], f32, tag="ot")
                    nc.vector.tensor_scalar_mul(ot[:], d1[:], rc_sb[:, j : j + 1])
                    ow = nc.sync.dma_start(seg_out[b, bass.ts(j, 128), :], ot[:])
                    add_dep_helper(_mi(ow), _mi(last_load), sync=False,
                                   reason="out writes after all loads on sync ring")

    nc.compile()
    return nc


def _get_program():
    if "nc" not in _CACHE:
        _CACHE["nc"] = _build_program()
    return _CACHE["nc"]


def _host_prep(asr_alignment, text_token_len):
    import ml_dtypes
    a = np.asarray(asr_alignment).astype(np.int64)
    s, e = a[..., 0], a[..., 1]
    tlen = np.asarray(text_token_len).astype(np.int64)
    tmask = np.arange(TT)[None, :] < tlen[:, None]

    pe = np.where(((e + 1) % 128) != 0, e, ZROW)
    ps = np.where((s % 128) != 0, s - 1, ZROW)
    pe = np.where(tmask, pe, ZROW)
    ps = np.where(tmask, ps, ZROW)
    be = np.where(tmask, (e + 1) >> 7, 0)
    bs = np.where(tmask, s >> 7, 0)

    cnt = (e - s + 1).astype(np.float64)
    recip = np.where(tmask, 1.0 / cnt, 0.0).astype(np.float32)

    # dma_gather wrap layout per 512-token half: flat token k at
    # [k % 16, k // 16], replicated across the 8 Q7-core partition groups.
    # Column blocks: [pe half0 | pe half1 | ps half0 | ps half1]
    def wrap(v):  # v: [B, 512] -> [B, 128, 32] int16
        w = v.reshape(B, 32, 16).transpose(0, 2, 1).astype(np.int16)
        return np.tile(w, (1, 8, 1))

    H = TT // 2
    pidx = np.concatenate([wrap(pe[:, :H]), wrap(pe[:, H:]),
                           wrap(ps[:, :H]), wrap(ps[:, H:])], axis=2)
    pidx = np.ascontiguousarray(pidx)

    k = np.arange(128)
    ut_c = (k[:, None] <= k[None, :]).astype(ml_dtypes.bfloat16)
    j32 = np.arange(NBLK)[:, None]
    stut_c = (j32 < np.arange(POFF)[None, :]).astype(np.float32)

    ks = np.arange(POFF)[None, :, None]
    bt = ((ks == be[:, None, :]).astype(np.float32)
          - (ks == bs[:, None, :]).astype(np.float32)).astype(ml_dtypes.bfloat16)

    recip_dev = recip.reshape(B, NTOK, 128).transpose(0, 2, 1)
    recip_dev = np.ascontiguousarray(recip_dev, np.float32)
    return pidx, np.ascontiguousarray(bt), recip_dev, ut_c, stut_c


def _run(inputs_by_core, trace=False, **kw):
    from concourse.bass_utils import run_bass_kernel_spmd
    nc = _get_program()
    return run_bass_kernel_spmd(nc, inputs_by_core,
                                core_ids=list(range(NCORES)), trace=trace, **kw)


def _make_in_maps(audio_feats, asr_alignment, text_token_len):
    import ml_dtypes
    audio = np.ascontiguousarray(np.asarray(audio_feats), np.float32)
    audio_hl = np.empty((B, TA, 2 * D), ml_dtypes.bfloat16)
    audio_hl[:, :, :D] = audio.astype(ml_dtypes.bfloat16)
    audio_hl[:, :, D:] = (audio - audio_hl[:, :, :D].astype(np.float32)
                          ).astype(ml_dtypes.bfloat16)
    pidx, bt, recip, ut_c, stut_c = _host_prep(asr_alignment, text_token_len)
    lridx = (np.arange(NBLK, dtype=np.int32) * 128 + 127).reshape(NBLK, 1)
    in_maps = []
    for c in range(NCORES):
        sl = slice(BPC * c, BPC * (c + 1))
        in_maps.append({
            "audio_hl": audio_hl[sl],
            "pidx": pidx[sl],
            "bt": bt[sl],
            "recip": recip[sl],
            "lridx": lridx,
            "ut_c": ut_c,
            "stut_c": stut_c,
        })
    return in_maps


def kernel(audio_feats, audio_feats_len, text_token_for_audio,
           text_token_embed_for_audio, text_token_len, asr_alignment,
           _trace=False, **_kw):
    in_maps = _make_in_maps(audio_feats, asr_alignment, text_token_len)
    res = _run(in_maps, trace=_trace, **_kw)
    seg = np.concatenate([res.results[c]["seg"] for c in range(NCORES)], axis=0)
    out_len = np.asarray(text_token_len).astype(np.int32, copy=False)
    if _trace:
        return (seg, out_len), res
    return seg, out_len
